# revision 78
# baseline (speedup 1.0000x reference)
"""Causal self-attention (B=2, T=2048, D=1024, H=16, Dh=64) on 8 Trainium2 cores.

Sharding: (batch, head-group) — core c handles batch c//4 and heads 4*(c%4)..+4.
Each core computes Q/K/V projections for its 4 heads, causal attention, and a
partial output projection (its head-columns of Wo); the host sums the 4 partial
outputs per batch and adds bo.

All PE operands are fp16 (PSUM accumulation stays fp32): halves input DMA
bytes for a fast start, keeps ~0.05% elementwise precision, and permits exact
128-column causal trimming of the score / AV matmuls (the exp range is safe:
scores stay well under fp16 overflow).  Softmax denominators ride along as 64
replicated ones-columns in the AV stationary operand, so normalization is a
direct DVE reciprocal + multiply (no PE broadcast).  Attention runs as two
j-interleaved step streams (head0 then head2, head1 then head3, offset by two
steps) over 512-query chunks, so each head's exp latency and softmax tail hide
under the other stream's matmuls; projection and output-projection work is
emitted just-in-time as small "fill" units whose emission position (= Tile
scheduler priority) slots them into the ACT-bound stalls of the attention
pipeline.  NOTE: Tile tracks dependencies in emission order, so every fill
trigger must sit at or after the step that writes what the fill reads.

Per-core layouts:
  xs    [128, 8, T]       x[b] transposed, d on partitions (8 chunks of 128)
  qT/kT [128, 2, T]       head-major projections; partitions = 2 heads x 64 dims
  v_s   [128, 16, 4, 128] keys on partitions; per (t-block, head): 64 V columns
                          + 64 ones columns (softmax denominator replicas)
  S^T   [128, 1024] psum  scores transposed per 128-key block, exact causal trim
  yts   [128, 512] psum   rows 0:63 = unnormalized y^T, 64:127 = denominator
"""
import numpy as np

import concourse.bacc as bacc
import concourse.mybir as mybir
import concourse.tile as tile
from concourse.bass_utils import run_bass_kernel_spmd

F32 = mybir.dt.float32
F16 = mybir.dt.float16

B, T, D = 2, 2048, 1024
NH_LOC, DH = 4, 64          # heads per core, head dim
M = NH_LOC * DH             # 256 local qkv dims
KD = D // 128               # 8 contraction chunks
NT = T // 128               # 16 t-blocks
NC = T // 512               # 4 512-chunks
Exp = mybir.ActivationFunctionType.Exp


def _build():
    nc = bacc.Bacc("TRN2", target_bir_lowering=False, debug=False, num_devices=8)

    xT = nc.dram_tensor("xT", [D, T], F16, kind="ExternalInput")
    wqT = nc.dram_tensor("wqT", [D, M], F16, kind="ExternalInput")
    wkT = nc.dram_tensor("wkT", [D, M], F16, kind="ExternalInput")
    wvT = nc.dram_tensor("wvT", [D, M], F16, kind="ExternalInput")
    bq = nc.dram_tensor("bq", [M], F32, kind="ExternalInput")
    bk = nc.dram_tensor("bk", [M], F32, kind="ExternalInput")
    bv = nc.dram_tensor("bv", [M], F16, kind="ExternalInput")
    woT = nc.dram_tensor("woT", [M, D], F16, kind="ExternalInput")
    outp = nc.dram_tensor("outp", [T, D], F16, kind="ExternalOutput")

    with tile.TileContext(nc) as tc:
        with (
            tc.tile_pool(name="const", bufs=1) as const,
            tc.tile_pool(name="psS", bufs=3, space="PSUM") as psS,
            tc.tile_pool(name="psY", bufs=3, space="PSUM") as psY,
            tc.tile_pool(name="psF", bufs=2, space="PSUM") as psF,
            tc.tile_pool(name="pch", bufs=6) as pch,
            tc.tile_pool(name="tails", bufs=4) as tails,
            tc.tile_pool(name="outs", bufs=8) as outs,
        ):
            # ---- Input DMAs, ordered by first use.  The model's DMA device is
            # serial across queues, so the critical prefix (wq halves, first x
            # columns) goes back-to-back on the sync queue; everything the
            # prefix doesn't need rides the scalar queue, later.
            xs = const.tile([128, KD, T], F16)
            xr = xT.rearrange("(dd p) t -> p dd t", p=128)
            wq_s = const.tile([128, KD, M], F16)
            wqr = wqT.rearrange("(dd p) m -> p dd m", p=128)
            nc.sync.dma_start(out=xs[:, :, 0:256], in_=xr[:, :, 0:256])
            nc.sync.dma_start(out=wq_s[:, 0:2, :], in_=wqr[:, 0:2, :])
            nc.sync.dma_start(out=wq_s[:, 2:8, :], in_=wqr[:, 2:8, :])
            nc.sync.dma_start(out=xs[:, :, 256:512], in_=xr[:, :, 256:512])
            wk_s = const.tile([128, KD, M], F16)
            nc.sync.dma_start(out=wk_s, in_=wkT.rearrange("(dd p) m -> p dd m", p=128))
            wv_s = const.tile([128, KD, M], F16)
            nc.sync.dma_start(out=wv_s, in_=wvT.rearrange("(dd p) m -> p dd m", p=128))
            nc.sync.dma_start(out=xs[:, :, 512:1024], in_=xr[:, :, 512:1024])
            bq_s = const.tile([128, 2], F32)
            nc.scalar.dma_start(out=bq_s, in_=bq.rearrange("(mt p) -> p mt", p=128))
            bk_s = const.tile([128, 2], F32)
            nc.scalar.dma_start(out=bk_s, in_=bk.rearrange("(mt p) -> p mt", p=128))
            bv_row = const.tile([1, M], F16)
            nc.scalar.dma_start(out=bv_row, in_=bv[None, :])
            for c in range(2, NC):
                nc.sync.dma_start(
                    out=xs[:, :, c * 512:(c + 1) * 512], in_=xr[:, :, c * 512:(c + 1) * 512]
                )
            wo_s = const.tile([128, 2, D], F16)
            nc.sync.dma_start(out=wo_s, in_=woT.rearrange("(kk p) j -> p kk j", p=128))

            ones_t = const.tile([1, 128], F16)
            nc.vector.memset(ones_t, 1.0)

            qT_s = const.tile([128, 2, T], F16)
            kT_s = const.tile([128, 2, T], F16)
            yT_s = const.tile([128, 2, T], F16)
            v_s = const.tile([128, NT, NH_LOC, 2 * DH], F16)
            # ones columns 64:128 -> denominator replicas out of the AV matmul
            nc.gpsimd.memset(v_s[:, :, :, DH:2 * DH], 1.0)

            out_r = outp.rearrange("(tb p) j -> tb p j", p=128)

            def proj_qk_unit(w_s, b_s, dst, mt, c0, width):
                pp = psF.tile([128, 512], F32, tag="fill")
                for dd in range(KD):
                    nc.tensor.matmul(
                        pp[:, 0:width],
                        w_s[:, dd, mt * 128:(mt + 1) * 128],
                        xs[:, dd, c0:c0 + width],
                        start=(dd == 0), stop=(dd == KD - 1),
                    )
                nc.vector.tensor_scalar_add(
                    dst[:, mt, c0:c0 + width], pp[:, 0:width], b_s[:, mt:mt + 1]
                )

            def proj_v_unit(tb):
                pv = psF.tile([128, 512], F32, tag="fill")
                for dd in range(KD):
                    nc.tensor.matmul(
                        pv[:, 0:M],
                        xs[:, dd, tb * 128:(tb + 1) * 128],
                        wv_s[:, dd, :],
                        start=(dd == 0), stop=False,
                    )
                nc.tensor.matmul(pv[:, 0:M], ones_t, bv_row, start=False, stop=True)
                nc.any.tensor_copy(
                    v_s[:, tb, :, 0:DH],
                    pv[:, 0:M].rearrange("p (h d) -> p h d", h=NH_LOC),
                )

            def oproj_unit(tb, n, last=False):
                po = psF.tile([128, 512], F32, tag="fill")
                for kk in range(2):
                    nc.tensor.matmul(
                        po,
                        yT_s[:, kk, tb * 128:(tb + 1) * 128],
                        wo_s[:, kk, n * 512:(n + 1) * 512],
                        start=(kk == 0), stop=(kk == 1),
                    )
                o_sb = outs.tile([128, 512], F16, tag="o")
                # tail units split copy/DMA across engines/queues to drain fast
                if last and (tb + n) % 2 == 1:
                    nc.scalar.copy(o_sb, po)
                    nc.scalar.dma_start(
                        out=out_r[tb][:, n * 512:(n + 1) * 512], in_=o_sb
                    )
                else:
                    nc.any.tensor_copy(o_sb, po)
                    nc.sync.dma_start(
                        out=out_r[tb][:, n * 512:(n + 1) * 512], in_=o_sb
                    )

            def tail(h, yt, c):
                # normalize chunk c: denominator replicas live in rows 64:128;
                # 256-column halves so dependent output projections start early
                for u in range(2):
                    sl = slice(256 * u, 256 * (u + 1))
                    rec = tails.tile([64, 256], F32, tag="rec")
                    nc.vector.reciprocal(rec, yt[64:128, sl])
                    nc.any.tensor_mul(
                        yT_s[(h % 2) * 64:(h % 2) * 64 + 64, h // 2,
                             c * 512 + 256 * u:c * 512 + 256 * (u + 1)],
                        yt[0:64, sl], rec,
                    )

            def qk_exp_av(h, c, j, yt):
                # one key block j of chunk c (queries [512c, 512c+512))
                po, mt = (h % 2) * 64, h // 2
                lo = max(0, j * 128 - 512 * c)   # causal left trim
                st = psS.tile([128, 512], F32, tag="st")
                p_ch = pch.tile([128, 512], F16, tag="p")
                nc.tensor.matmul(
                    st[:, lo:512],
                    kT_s[po:po + 64, mt, j * 128:(j + 1) * 128],
                    qT_s[po:po + 64, mt, 512 * c + lo:512 * (c + 1)],
                    start=True, stop=True,
                )
                nc.scalar.activation(p_ch[:, lo:512], st[:, lo:512], Exp)
                if j * 128 >= 512 * c:
                    # zero the upper triangle of the 128-wide diagonal block:
                    # keep iff query_col >= key_row
                    nc.gpsimd.affine_select(
                        out=p_ch[:, lo:lo + 128], in_=p_ch[:, lo:lo + 128],
                        compare_op=mybir.AluOpType.is_ge, fill=0.0,
                        base=0, channel_multiplier=-1, pattern=[[1, 128]],
                    )
                nc.tensor.matmul(
                    yt[:, lo:512],
                    v_s[:, j, h, :],
                    p_ch[:, lo:512],
                    start=(j == 0), stop=(j == 4 * c + 3),
                )
                if j == 4 * c + 3:
                    tail(h, yt, c)

            # ---- Emission order (= scheduler priority) ----
            # All four heads form two long step streams (head0 then head2;
            # head1 then head3) merged round-robin with stream 2 offset by two
            # steps, so no two chunk/phase boundaries coincide: each head's
            # softmax-tail drought is hidden by the other stream mid-chunk.
            # Heads 0,1 run chunks ascending (chunk c only needs projections
            # through column 512(c+1)); heads 2,3 run descending so the
            # per-chunk output projections they unlock fill the later chunks.
            s1 = [(0, c, j) for c in (0, 1, 2, 3) for j in range(4 * c + 4)] \
               + [(2, c, j) for c in (3, 2, 1, 0) for j in range(4 * c + 4)]
            s2 = [(1, c, j) for c in (0, 1, 2, 3) for j in range(4 * c + 4)] \
               + [(3, c, j) for c in (3, 2, 1, 0) for j in range(4 * c + 4)]
            merged = [s1[0], s1[1]]
            for a, b in zip(s1[2:], s2):
                merged += [a, b]
            merged += s2[-2:]

            def proj_chunk(c):
                w = 256 if c == 0 else 512
                for c0 in range(c * 512, (c + 1) * 512, w):
                    proj_qk_unit(wq_s, bq_s, qT_s, 0, c0, w)
                for c0 in range(c * 512, (c + 1) * 512, w):
                    proj_qk_unit(wk_s, bk_s, kT_s, 0, c0, w)
                for tb in range(4 * c, 4 * c + 4):
                    proj_v_unit(tb)

            # fill units queued at trigger steps (emitted right after them)
            fills = {
                (0, 0, 1): lambda: proj_chunk(1),
                (0, 1, 1): lambda: proj_chunk(2),
                (0, 2, 5): lambda: proj_chunk(3),
                (0, 3, 7): lambda: (
                    proj_qk_unit(wk_s, bk_s, kT_s, 1, 0, 512),
                    proj_qk_unit(wq_s, bq_s, qT_s, 1, 1536, 512),
                ),
                (0, 3, 13): lambda:
                    proj_qk_unit(wk_s, bk_s, kT_s, 1, 512, 512),
                (2, 3, 3): lambda:
                    proj_qk_unit(wk_s, bk_s, kT_s, 1, 1024, 512),
                (2, 3, 7): lambda:
                    proj_qk_unit(wk_s, bk_s, kT_s, 1, 1536, 512),
                (2, 3, 11): lambda:
                    proj_qk_unit(wq_s, bq_s, qT_s, 1, 1024, 512),
                (2, 3, 15): lambda:
                    proj_qk_unit(wq_s, bq_s, qT_s, 1, 512, 512),
                (2, 2, 7): lambda:
                    proj_qk_unit(wq_s, bq_s, qT_s, 1, 0, 512),
            }
            # output projections for chunk c right after head 3's chunk-c tail
            for c in range(NC):
                def op(c=c):
                    for tb in range(4 * c, 4 * c + 4):
                        for n in range(2):
                            oproj_unit(tb, n, last=(c == 0))
                fills[(3, c, 4 * c + 3)] = op

            proj_chunk(0)
            yts = {}
            for (h, c, j) in merged:
                if j == 0:
                    yts[h] = psY.tile([128, 512], F32, tag="yt", name=f"yt_{h}_{c}")
                qk_exp_av(h, c, j, yts[h])
                f = fills.pop((h, c, j), None)
                if f is not None:
                    f()
            assert not fills, f"unfired fill triggers: {list(fills)}"

    nc.compile()
    return nc


_NC = None


def _get_nc():
    global _NC
    if _NC is None:
        _NC = _build()
    return _NC


def kernel(x, Wq, bq, Wk, bk, Wv, bv, Wo, bo, _trace=False):
    x = np.asarray(x, dtype=np.float32)
    Wq = np.asarray(Wq, dtype=np.float32)
    Wk = np.asarray(Wk, dtype=np.float32)
    Wv = np.asarray(Wv, dtype=np.float32)
    Wo = np.asarray(Wo, dtype=np.float32)
    bq = np.asarray(bq, dtype=np.float32)
    bk = np.asarray(bk, dtype=np.float32)
    bv = np.asarray(bv, dtype=np.float32)
    bo = np.asarray(bo, dtype=np.float32)

    scale = np.float32(1.0 / np.sqrt(DH))
    bf = np.float16
    in_maps = []
    for c in range(8):
        b, roff = c // 4, (c % 4) * M
        in_maps.append({
            "xT": np.ascontiguousarray(x[b].T).astype(bf),
            "wqT": np.ascontiguousarray((Wq[roff:roff + M] * scale).T).astype(bf),
            "wkT": np.ascontiguousarray(Wk[roff:roff + M].T).astype(bf),
            "wvT": np.ascontiguousarray(Wv[roff:roff + M].T).astype(bf),
            "bq": np.ascontiguousarray(bq[roff:roff + M] * scale),
            "bk": np.ascontiguousarray(bk[roff:roff + M]),
            "bv": np.ascontiguousarray(bv[roff:roff + M]).astype(bf),
            "woT": np.ascontiguousarray(Wo[:, roff:roff + M].T).astype(bf),
        })

    nc = _get_nc()
    res = run_bass_kernel_spmd(nc, in_maps, list(range(8)), trace=_trace)

    out = np.empty((B, T, D), dtype=np.float32)
    for b in range(B):
        acc = np.zeros((T, D), dtype=np.float64)
        for c in range(4 * b, 4 * b + 4):
            acc += res.results[c]["outp"]
        out[b] = (acc + bo.astype(np.float64)).astype(np.float32)
    if _trace:
        kernel.last_results = res
    return out


# revision 85
# speedup vs baseline: 1.0106x; 1.0106x over previous
"""Causal self-attention (B=2, T=2048, D=1024, H=16, Dh=64) on 8 Trainium2 cores.

Sharding: (batch, head-group) — core c handles batch c//4 and heads 4*(c%4)..+4.
Each core computes Q/K/V projections for its 4 heads, causal attention, and a
partial output projection (its head-columns of Wo); the host sums the 4 partial
outputs per batch and adds bo.

All PE operands are fp16 (PSUM accumulation stays fp32): halves input DMA
bytes for a fast start, keeps ~0.05% elementwise precision, and permits exact
128-column causal trimming of the score / AV matmuls (the exp range is safe:
scores stay well under fp16 overflow).  Softmax denominators ride along as 64
replicated ones-columns in the AV stationary operand, so normalization is a
direct DVE reciprocal + multiply (no PE broadcast).  Attention runs as two
j-interleaved step streams (head0 then head2, head1 then head3, offset by two
steps) over 512-query chunks, so each head's exp latency and softmax tail hide
under the other stream's matmuls; projection and output-projection work is
emitted just-in-time as small "fill" units whose emission position (= Tile
scheduler priority) slots them into the ACT-bound stalls of the attention
pipeline.  NOTE: Tile tracks dependencies in emission order, so every fill
trigger must sit at or after the step that writes what the fill reads.

Per-core layouts:
  xs    [128, 8, T]       x[b] transposed, d on partitions (8 chunks of 128)
  qT/kT [128, 2, T]       head-major projections; partitions = 2 heads x 64 dims
  v_s   [128, 16, 4, 128] keys on partitions; per (t-block, head): 64 V columns
                          + 64 ones columns (softmax denominator replicas)
  S^T   [128, 1024] psum  scores transposed per 128-key block, exact causal trim
  yts   [128, 512] psum   rows 0:63 = unnormalized y^T, 64:127 = denominator
"""
import numpy as np

import concourse.bacc as bacc
import concourse.mybir as mybir
import concourse.tile as tile
from concourse.bass_utils import run_bass_kernel_spmd

F32 = mybir.dt.float32
F16 = mybir.dt.float16

B, T, D = 2, 2048, 1024
NH_LOC, DH = 4, 64          # heads per core, head dim
M = NH_LOC * DH             # 256 local qkv dims
KD = D // 128               # 8 contraction chunks
NT = T // 128               # 16 t-blocks
NC = T // 512               # 4 512-chunks
Exp = mybir.ActivationFunctionType.Exp


def _build():
    nc = bacc.Bacc("TRN2", target_bir_lowering=False, debug=False, num_devices=8)

    xT = nc.dram_tensor("xT", [D, T], F16, kind="ExternalInput")
    wqT = nc.dram_tensor("wqT", [D, M], F16, kind="ExternalInput")
    wkT = nc.dram_tensor("wkT", [D, M], F16, kind="ExternalInput")
    wvT = nc.dram_tensor("wvT", [D, M], F16, kind="ExternalInput")
    bq = nc.dram_tensor("bq", [M], F32, kind="ExternalInput")
    bk = nc.dram_tensor("bk", [M], F32, kind="ExternalInput")
    bv = nc.dram_tensor("bv", [M], F16, kind="ExternalInput")
    woT = nc.dram_tensor("woT", [M, D], F16, kind="ExternalInput")
    outp = nc.dram_tensor("outp", [T, D], F16, kind="ExternalOutput")

    with tile.TileContext(nc) as tc:
        with (
            tc.tile_pool(name="const", bufs=1) as const,
            tc.tile_pool(name="psS", bufs=3, space="PSUM") as psS,
            tc.tile_pool(name="psY", bufs=3, space="PSUM") as psY,
            tc.tile_pool(name="psF", bufs=2, space="PSUM") as psF,
            tc.tile_pool(name="pch", bufs=6) as pch,
            tc.tile_pool(name="tails", bufs=4) as tails,
            tc.tile_pool(name="outs", bufs=8) as outs,
        ):
            # ---- Input DMAs, ordered by first use.  The model's DMA device is
            # serial across queues, so the critical prefix (wq halves, first x
            # columns) goes back-to-back on the sync queue; everything the
            # prefix doesn't need rides the scalar queue, later.
            xs = const.tile([128, KD, T], F16)
            xr = xT.rearrange("(dd p) t -> p dd t", p=128)
            wq_s = const.tile([128, KD, M], F16)
            wqr = wqT.rearrange("(dd p) m -> p dd m", p=128)
            nc.sync.dma_start(out=xs[:, :, 0:256], in_=xr[:, :, 0:256])
            nc.sync.dma_start(out=wq_s[:, 0:2, :], in_=wqr[:, 0:2, :])
            nc.sync.dma_start(out=wq_s[:, 2:8, :], in_=wqr[:, 2:8, :])
            nc.sync.dma_start(out=xs[:, :, 256:512], in_=xr[:, :, 256:512])
            wk_s = const.tile([128, KD, M], F16)
            nc.sync.dma_start(out=wk_s, in_=wkT.rearrange("(dd p) m -> p dd m", p=128))
            wv_s = const.tile([128, KD, M], F16)
            nc.sync.dma_start(out=wv_s, in_=wvT.rearrange("(dd p) m -> p dd m", p=128))
            nc.sync.dma_start(out=xs[:, :, 512:1024], in_=xr[:, :, 512:1024])
            bq_s = const.tile([128, 2], F32)
            nc.scalar.dma_start(out=bq_s, in_=bq.rearrange("(mt p) -> p mt", p=128))
            bk_s = const.tile([128, 2], F32)
            nc.scalar.dma_start(out=bk_s, in_=bk.rearrange("(mt p) -> p mt", p=128))
            bv_row = const.tile([1, M], F16)
            nc.scalar.dma_start(out=bv_row, in_=bv[None, :])
            for c in range(2, NC):
                nc.sync.dma_start(
                    out=xs[:, :, c * 512:(c + 1) * 512], in_=xr[:, :, c * 512:(c + 1) * 512]
                )
            wo_s = const.tile([128, 2, D], F16)
            nc.sync.dma_start(out=wo_s, in_=woT.rearrange("(kk p) j -> p kk j", p=128))

            ones_t = const.tile([1, 128], F16)
            nc.vector.memset(ones_t, 1.0)

            qT_s = const.tile([128, 2, T], F16)
            kT_s = const.tile([128, 2, T], F16)
            yT_s = const.tile([128, 2, T], F16)
            v_s = const.tile([128, NT, NH_LOC, 2 * DH], F16)
            # ones columns 64:128 -> denominator replicas out of the AV matmul
            nc.gpsimd.memset(v_s[:, :, :, DH:2 * DH], 1.0)

            out_r = outp.rearrange("(tb p) j -> tb p j", p=128)

            def proj_qk_unit(w_s, b_s, dst, mt, c0, width):
                pp = psF.tile([128, 512], F32, tag="fill")
                for dd in range(KD):
                    nc.tensor.matmul(
                        pp[:, 0:width],
                        w_s[:, dd, mt * 128:(mt + 1) * 128],
                        xs[:, dd, c0:c0 + width],
                        start=(dd == 0), stop=(dd == KD - 1),
                    )
                nc.vector.tensor_scalar_add(
                    dst[:, mt, c0:c0 + width], pp[:, 0:width], b_s[:, mt:mt + 1]
                )

            def proj_v_unit(tb):
                pv = psF.tile([128, 512], F32, tag="fill")
                for dd in range(KD):
                    nc.tensor.matmul(
                        pv[:, 0:M],
                        xs[:, dd, tb * 128:(tb + 1) * 128],
                        wv_s[:, dd, :],
                        start=(dd == 0), stop=False,
                    )
                nc.tensor.matmul(pv[:, 0:M], ones_t, bv_row, start=False, stop=True)
                nc.any.tensor_copy(
                    v_s[:, tb, :, 0:DH],
                    pv[:, 0:M].rearrange("p (h d) -> p h d", h=NH_LOC),
                )

            def oproj_unit(tb, n, last=False):
                po = psF.tile([128, 512], F32, tag="fill")
                for kk in range(2):
                    nc.tensor.matmul(
                        po,
                        yT_s[:, kk, tb * 128:(tb + 1) * 128],
                        wo_s[:, kk, n * 512:(n + 1) * 512],
                        start=(kk == 0), stop=(kk == 1),
                    )
                o_sb = outs.tile([128, 512], F16, tag="o")
                # tail units split copy/DMA across engines/queues to drain fast
                if last and (tb + n) % 2 == 1:
                    nc.scalar.copy(o_sb, po)
                    nc.scalar.dma_start(
                        out=out_r[tb][:, n * 512:(n + 1) * 512], in_=o_sb
                    )
                else:
                    nc.any.tensor_copy(o_sb, po)
                    nc.sync.dma_start(
                        out=out_r[tb][:, n * 512:(n + 1) * 512], in_=o_sb
                    )

            def tail(h, yt, c):
                # normalize chunk c: denominator replicas live in rows 64:128;
                # 256-column halves so dependent output projections start early
                for u in range(2):
                    sl = slice(256 * u, 256 * (u + 1))
                    rec = tails.tile([64, 256], F32, tag="rec")
                    nc.vector.reciprocal(rec, yt[64:128, sl])
                    nc.any.tensor_mul(
                        yT_s[(h % 2) * 64:(h % 2) * 64 + 64, h // 2,
                             c * 512 + 256 * u:c * 512 + 256 * (u + 1)],
                        yt[0:64, sl], rec,
                    )

            def qk_exp_av(h, c, j, yt):
                # one key block j of chunk c (queries [512c, 512c+512))
                po, mt = (h % 2) * 64, h // 2
                lo = max(0, j * 128 - 512 * c)   # causal left trim
                st = psS.tile([128, 512], F32, tag="st")
                p_ch = pch.tile([128, 512], F16, tag="p")
                nc.tensor.matmul(
                    st[:, lo:512],
                    kT_s[po:po + 64, mt, j * 128:(j + 1) * 128],
                    qT_s[po:po + 64, mt, 512 * c + lo:512 * (c + 1)],
                    start=True, stop=True,
                )
                nc.scalar.activation(p_ch[:, lo:512], st[:, lo:512], Exp)
                if j * 128 >= 512 * c:
                    # zero the upper triangle of the 128-wide diagonal block:
                    # keep iff query_col >= key_row
                    nc.gpsimd.affine_select(
                        out=p_ch[:, lo:lo + 128], in_=p_ch[:, lo:lo + 128],
                        compare_op=mybir.AluOpType.is_ge, fill=0.0,
                        base=0, channel_multiplier=-1, pattern=[[1, 128]],
                    )
                nc.tensor.matmul(
                    yt[:, lo:512],
                    v_s[:, j, h, :],
                    p_ch[:, lo:512],
                    start=(j == 0), stop=(j == 4 * c + 3),
                )
                if j == 4 * c + 3:
                    tail(h, yt, c)

            # ---- Emission order (= scheduler priority) ----
            # All four heads form two long step streams (head0 then head2;
            # head1 then head3) merged round-robin with stream 2 offset by two
            # steps, so no two chunk/phase boundaries coincide: each head's
            # softmax-tail drought is hidden by the other stream mid-chunk.
            # Heads 0,1 run chunks ascending (chunk c only needs projections
            # through column 512(c+1)); heads 2,3 run descending so the
            # per-chunk output projections they unlock fill the later chunks.
            s1 = [(0, c, j) for c in (0, 1, 2, 3) for j in range(4 * c + 4)] \
               + [(2, c, j) for c in (3, 2, 1, 0) for j in range(4 * c + 4)]
            s2 = [(1, c, j) for c in (0, 1, 2, 3) for j in range(4 * c + 4)] \
               + [(3, c, j) for c in (3, 2, 1, 0) for j in range(4 * c + 4)]
            merged = [s1[0], s1[1]]
            for a, b in zip(s1[2:], s2):
                merged += [a, b]
            merged += s2[-2:]

            def proj_chunk(c):
                w = 256 if c == 0 else 512
                for c0 in range(c * 512, (c + 1) * 512, w):
                    proj_qk_unit(wq_s, bq_s, qT_s, 0, c0, w)
                for c0 in range(c * 512, (c + 1) * 512, w):
                    proj_qk_unit(wk_s, bk_s, kT_s, 0, c0, w)
                for tb in range(4 * c, 4 * c + 4):
                    proj_v_unit(tb)

            # fill units queued at trigger steps (emitted right after them)
            fills = {
                (0, 0, 1): lambda: proj_chunk(1),
                (0, 1, 1): lambda: proj_chunk(2),
                (0, 2, 11): lambda: proj_chunk(3),
                (0, 3, 7): lambda: (
                    proj_qk_unit(wk_s, bk_s, kT_s, 1, 0, 512),
                    proj_qk_unit(wq_s, bq_s, qT_s, 1, 1536, 512),
                ),
                (0, 3, 13): lambda:
                    proj_qk_unit(wk_s, bk_s, kT_s, 1, 512, 512),
                (2, 3, 3): lambda:
                    proj_qk_unit(wk_s, bk_s, kT_s, 1, 1024, 512),
                (2, 3, 7): lambda:
                    proj_qk_unit(wk_s, bk_s, kT_s, 1, 1536, 512),
                (2, 3, 11): lambda:
                    proj_qk_unit(wq_s, bq_s, qT_s, 1, 1024, 512),
                (2, 3, 15): lambda:
                    proj_qk_unit(wq_s, bq_s, qT_s, 1, 512, 512),
                (2, 2, 7): lambda:
                    proj_qk_unit(wq_s, bq_s, qT_s, 1, 0, 512),
            }
            # output projections for chunk c right after head 3's chunk-c tail
            for c in range(NC):
                def op(c=c):
                    for tb in range(4 * c, 4 * c + 4):
                        for n in range(2):
                            oproj_unit(tb, n, last=(c == 0))
                fills[(3, c, 4 * c + 3)] = op

            proj_chunk(0)
            yts = {}
            for (h, c, j) in merged:
                if j == 0:
                    yts[h] = psY.tile([128, 512], F32, tag="yt", name=f"yt_{h}_{c}")
                qk_exp_av(h, c, j, yts[h])
                f = fills.pop((h, c, j), None)
                if f is not None:
                    f()
            assert not fills, f"unfired fill triggers: {list(fills)}"

    nc.compile()
    return nc


_NC = None


def _get_nc():
    global _NC
    if _NC is None:
        _NC = _build()
    return _NC


def kernel(x, Wq, bq, Wk, bk, Wv, bv, Wo, bo, _trace=False):
    x = np.asarray(x, dtype=np.float32)
    Wq = np.asarray(Wq, dtype=np.float32)
    Wk = np.asarray(Wk, dtype=np.float32)
    Wv = np.asarray(Wv, dtype=np.float32)
    Wo = np.asarray(Wo, dtype=np.float32)
    bq = np.asarray(bq, dtype=np.float32)
    bk = np.asarray(bk, dtype=np.float32)
    bv = np.asarray(bv, dtype=np.float32)
    bo = np.asarray(bo, dtype=np.float32)

    scale = np.float32(1.0 / np.sqrt(DH))
    bf = np.float16
    in_maps = []
    for c in range(8):
        b, roff = c // 4, (c % 4) * M
        in_maps.append({
            "xT": np.ascontiguousarray(x[b].T).astype(bf),
            "wqT": np.ascontiguousarray((Wq[roff:roff + M] * scale).T).astype(bf),
            "wkT": np.ascontiguousarray(Wk[roff:roff + M].T).astype(bf),
            "wvT": np.ascontiguousarray(Wv[roff:roff + M].T).astype(bf),
            "bq": np.ascontiguousarray(bq[roff:roff + M] * scale),
            "bk": np.ascontiguousarray(bk[roff:roff + M]),
            "bv": np.ascontiguousarray(bv[roff:roff + M]).astype(bf),
            "woT": np.ascontiguousarray(Wo[:, roff:roff + M].T).astype(bf),
        })

    nc = _get_nc()
    res = run_bass_kernel_spmd(nc, in_maps, list(range(8)), trace=_trace)

    out = np.empty((B, T, D), dtype=np.float32)
    for b in range(B):
        acc = np.zeros((T, D), dtype=np.float64)
        for c in range(4 * b, 4 * b + 4):
            acc += res.results[c]["outp"]
        out[b] = (acc + bo.astype(np.float64)).astype(np.float32)
    if _trace:
        kernel.last_results = res
    return out


# revision 88
# speedup vs baseline: 1.0180x; 1.0073x over previous
"""Causal self-attention (B=2, T=2048, D=1024, H=16, Dh=64) on 8 Trainium2 cores.

Sharding: (batch, head-group) — core c handles batch c//4 and heads 4*(c%4)..+4.
Each core computes Q/K/V projections for its 4 heads, causal attention, and a
partial output projection (its head-columns of Wo); the host sums the 4 partial
outputs per batch and adds bo.

All PE operands are fp16 (PSUM accumulation stays fp32): halves input DMA
bytes for a fast start, keeps ~0.05% elementwise precision, and permits exact
128-column causal trimming of the score / AV matmuls (the exp range is safe:
scores stay well under fp16 overflow).  Softmax denominators ride along as 64
replicated ones-columns in the AV stationary operand, so normalization is a
direct DVE reciprocal + multiply (no PE broadcast).  Attention runs as two
j-interleaved step streams (head0 then head2, head1 then head3, offset by two
steps) over 512-query chunks, so each head's exp latency and softmax tail hide
under the other stream's matmuls; projection and output-projection work is
emitted just-in-time as small "fill" units whose emission position (= Tile
scheduler priority) slots them into the ACT-bound stalls of the attention
pipeline.  NOTE: Tile tracks dependencies in emission order, so every fill
trigger must sit at or after the step that writes what the fill reads.

Per-core layouts:
  xs    [128, 8, T]       x[b] transposed, d on partitions (8 chunks of 128)
  qT/kT [128, 2, T]       head-major projections; partitions = 2 heads x 64 dims
  v_s   [128, 16, 4, 128] keys on partitions; per (t-block, head): 64 V columns
                          + 64 ones columns (softmax denominator replicas)
  S^T   [128, 1024] psum  scores transposed per 128-key block, exact causal trim
  yts   [128, 512] psum   rows 0:63 = unnormalized y^T, 64:127 = denominator
"""
import numpy as np

import concourse.bacc as bacc
import concourse.mybir as mybir
import concourse.tile as tile
from concourse.bass_utils import run_bass_kernel_spmd

F32 = mybir.dt.float32
F16 = mybir.dt.float16

B, T, D = 2, 2048, 1024
NH_LOC, DH = 4, 64          # heads per core, head dim
M = NH_LOC * DH             # 256 local qkv dims
KD = D // 128               # 8 contraction chunks
NT = T // 128               # 16 t-blocks
NC = T // 512               # 4 512-chunks
Exp = mybir.ActivationFunctionType.Exp


def _build():
    nc = bacc.Bacc("TRN2", target_bir_lowering=False, debug=False, num_devices=8)

    xT = nc.dram_tensor("xT", [D, T], F16, kind="ExternalInput")
    wqT = nc.dram_tensor("wqT", [D, M], F16, kind="ExternalInput")
    wkT = nc.dram_tensor("wkT", [D, M], F16, kind="ExternalInput")
    wvT = nc.dram_tensor("wvT", [D, M], F16, kind="ExternalInput")
    bq = nc.dram_tensor("bq", [M], F32, kind="ExternalInput")
    bk = nc.dram_tensor("bk", [M], F32, kind="ExternalInput")
    bv = nc.dram_tensor("bv", [M], F16, kind="ExternalInput")
    woT = nc.dram_tensor("woT", [M, D], F16, kind="ExternalInput")
    outp = nc.dram_tensor("outp", [T, D], F16, kind="ExternalOutput")

    with tile.TileContext(nc) as tc:
        with (
            tc.tile_pool(name="const", bufs=1) as const,
            tc.tile_pool(name="psS", bufs=3, space="PSUM") as psS,
            tc.tile_pool(name="psY", bufs=3, space="PSUM") as psY,
            tc.tile_pool(name="psF", bufs=2, space="PSUM") as psF,
            tc.tile_pool(name="pch", bufs=6) as pch,
            tc.tile_pool(name="tails", bufs=4) as tails,
            tc.tile_pool(name="outs", bufs=8) as outs,
        ):
            # ---- Input DMAs, ordered by first use.  The model's DMA device is
            # serial across queues, so the critical prefix (wq halves, first x
            # columns) goes back-to-back on the sync queue; everything the
            # prefix doesn't need rides the scalar queue, later.
            xs = const.tile([128, KD, T], F16)
            xr = xT.rearrange("(dd p) t -> p dd t", p=128)
            wq_s = const.tile([128, KD, M], F16)
            wqr = wqT.rearrange("(dd p) m -> p dd m", p=128)
            nc.sync.dma_start(out=xs[:, :, 0:256], in_=xr[:, :, 0:256])
            nc.sync.dma_start(out=wq_s[:, 0:2, :], in_=wqr[:, 0:2, :])
            nc.sync.dma_start(out=wq_s[:, 2:8, :], in_=wqr[:, 2:8, :])
            nc.sync.dma_start(out=xs[:, :, 256:512], in_=xr[:, :, 256:512])
            wk_s = const.tile([128, KD, M], F16)
            nc.sync.dma_start(out=wk_s, in_=wkT.rearrange("(dd p) m -> p dd m", p=128))
            wv_s = const.tile([128, KD, M], F16)
            nc.sync.dma_start(out=wv_s, in_=wvT.rearrange("(dd p) m -> p dd m", p=128))
            nc.sync.dma_start(out=xs[:, :, 512:1024], in_=xr[:, :, 512:1024])
            bq_s = const.tile([128, 2], F32)
            nc.scalar.dma_start(out=bq_s, in_=bq.rearrange("(mt p) -> p mt", p=128))
            bk_s = const.tile([128, 2], F32)
            nc.scalar.dma_start(out=bk_s, in_=bk.rearrange("(mt p) -> p mt", p=128))
            bv_row = const.tile([1, M], F16)
            nc.scalar.dma_start(out=bv_row, in_=bv[None, :])
            for c in range(2, NC):
                nc.sync.dma_start(
                    out=xs[:, :, c * 512:(c + 1) * 512], in_=xr[:, :, c * 512:(c + 1) * 512]
                )
            wo_s = const.tile([128, 2, D], F16)
            nc.sync.dma_start(out=wo_s, in_=woT.rearrange("(kk p) j -> p kk j", p=128))

            ones_t = const.tile([1, 128], F16)
            nc.vector.memset(ones_t, 1.0)

            qT_s = const.tile([128, 2, T], F16)
            kT_s = const.tile([128, 2, T], F16)
            yT_s = const.tile([128, 2, T], F16)
            v_s = const.tile([128, NT, NH_LOC, 2 * DH], F16)
            # ones columns 64:128 -> denominator replicas out of the AV matmul
            nc.gpsimd.memset(v_s[:, :, :, DH:2 * DH], 1.0)

            out_r = outp.rearrange("(tb p) j -> tb p j", p=128)

            def proj_qk_unit(w_s, b_s, dst, mt, c0, width):
                pp = psF.tile([128, 512], F32, tag="fill")
                for dd in range(KD):
                    nc.tensor.matmul(
                        pp[:, 0:width],
                        w_s[:, dd, mt * 128:(mt + 1) * 128],
                        xs[:, dd, c0:c0 + width],
                        start=(dd == 0), stop=(dd == KD - 1),
                    )
                nc.vector.tensor_scalar_add(
                    dst[:, mt, c0:c0 + width], pp[:, 0:width], b_s[:, mt:mt + 1]
                )

            def proj_v_unit(tb):
                pv = psF.tile([128, 512], F32, tag="fill")
                for dd in range(KD):
                    nc.tensor.matmul(
                        pv[:, 0:M],
                        xs[:, dd, tb * 128:(tb + 1) * 128],
                        wv_s[:, dd, :],
                        start=(dd == 0), stop=False,
                    )
                nc.tensor.matmul(pv[:, 0:M], ones_t, bv_row, start=False, stop=True)
                nc.any.tensor_copy(
                    v_s[:, tb, :, 0:DH],
                    pv[:, 0:M].rearrange("p (h d) -> p h d", h=NH_LOC),
                )

            def oproj_unit(tb, n, last=False):
                po = psF.tile([128, 512], F32, tag="fill")
                for kk in range(2):
                    nc.tensor.matmul(
                        po,
                        yT_s[:, kk, tb * 128:(tb + 1) * 128],
                        wo_s[:, kk, n * 512:(n + 1) * 512],
                        start=(kk == 0), stop=(kk == 1),
                    )
                o_sb = outs.tile([128, 512], F16, tag="o")
                # tail units split copy/DMA across engines/queues to drain fast
                if last and (tb + n) % 2 == 1:
                    nc.scalar.copy(o_sb, po)
                    nc.scalar.dma_start(
                        out=out_r[tb][:, n * 512:(n + 1) * 512], in_=o_sb
                    )
                else:
                    nc.any.tensor_copy(o_sb, po)
                    nc.sync.dma_start(
                        out=out_r[tb][:, n * 512:(n + 1) * 512], in_=o_sb
                    )

            def tail(h, yt, c):
                # normalize chunk c: denominator replicas live in rows 64:128;
                # 256-column halves so dependent output projections start early
                for u in range(2):
                    sl = slice(256 * u, 256 * (u + 1))
                    rec = tails.tile([64, 256], F32, tag="rec")
                    nc.vector.reciprocal(rec, yt[64:128, sl])
                    nc.any.tensor_mul(
                        yT_s[(h % 2) * 64:(h % 2) * 64 + 64, h // 2,
                             c * 512 + 256 * u:c * 512 + 256 * (u + 1)],
                        yt[0:64, sl], rec,
                    )

            def qk_exp_av(h, c, j, yt):
                # one key block j of chunk c (queries [512c, 512c+512))
                po, mt = (h % 2) * 64, h // 2
                lo = max(0, j * 128 - 512 * c)   # causal left trim
                st = psS.tile([128, 512], F32, tag="st")
                p_ch = pch.tile([128, 512], F16, tag="p")
                nc.tensor.matmul(
                    st[:, lo:512],
                    kT_s[po:po + 64, mt, j * 128:(j + 1) * 128],
                    qT_s[po:po + 64, mt, 512 * c + lo:512 * (c + 1)],
                    start=True, stop=True,
                )
                nc.scalar.activation(p_ch[:, lo:512], st[:, lo:512], Exp)
                if j * 128 >= 512 * c:
                    # zero the upper triangle of the 128-wide diagonal block:
                    # keep iff query_col >= key_row
                    nc.gpsimd.affine_select(
                        out=p_ch[:, lo:lo + 128], in_=p_ch[:, lo:lo + 128],
                        compare_op=mybir.AluOpType.is_ge, fill=0.0,
                        base=0, channel_multiplier=-1, pattern=[[1, 128]],
                    )
                nc.tensor.matmul(
                    yt[:, lo:512],
                    v_s[:, j, h, :],
                    p_ch[:, lo:512],
                    start=(j == 0), stop=(j == 4 * c + 3),
                )
                if j == 4 * c + 3:
                    tail(h, yt, c)

            # ---- Emission order (= scheduler priority) ----
            # All four heads form two long step streams (head0 then head2;
            # head1 then head3) merged round-robin with stream 2 offset by two
            # steps, so no two chunk/phase boundaries coincide: each head's
            # softmax-tail drought is hidden by the other stream mid-chunk.
            # Heads 0,1 run chunks ascending (chunk c only needs projections
            # through column 512(c+1)); heads 2,3 run descending so the
            # per-chunk output projections they unlock fill the later chunks.
            s1 = [(0, c, j) for c in (0, 1, 2, 3) for j in range(4 * c + 4)] \
               + [(2, c, j) for c in (3, 2, 1, 0) for j in range(4 * c + 4)]
            s2 = [(1, c, j) for c in (0, 1, 2, 3) for j in range(4 * c + 4)] \
               + [(3, c, j) for c in (3, 2, 1, 0) for j in range(4 * c + 4)]
            merged = [s1[0], s1[1]]
            for a, b in zip(s1[2:], s2):
                merged += [a, b]
            merged += s2[-2:]

            def proj_chunk(c):
                w = 256 if c == 0 else 512
                for c0 in range(c * 512, (c + 1) * 512, w):
                    proj_qk_unit(wq_s, bq_s, qT_s, 0, c0, w)
                for c0 in range(c * 512, (c + 1) * 512, w):
                    proj_qk_unit(wk_s, bk_s, kT_s, 0, c0, w)
                for tb in range(4 * c, 4 * c + 4):
                    proj_v_unit(tb)

            # fill units queued at trigger steps (emitted right after them)
            fills = {
                (0, 0, 1): lambda: proj_chunk(1),
                (0, 1, 1): lambda: proj_chunk(2),
                (0, 2, 11): lambda: proj_chunk(3),
                (0, 3, 9): lambda: (
                    proj_qk_unit(wk_s, bk_s, kT_s, 1, 0, 512),
                    proj_qk_unit(wq_s, bq_s, qT_s, 1, 1536, 512),
                ),
                (0, 3, 15): lambda:
                    proj_qk_unit(wk_s, bk_s, kT_s, 1, 512, 512),
                (2, 3, 3): lambda:
                    proj_qk_unit(wk_s, bk_s, kT_s, 1, 1024, 512),
                (2, 3, 7): lambda:
                    proj_qk_unit(wk_s, bk_s, kT_s, 1, 1536, 512),
                (2, 3, 11): lambda:
                    proj_qk_unit(wq_s, bq_s, qT_s, 1, 1024, 512),
                (2, 3, 15): lambda:
                    proj_qk_unit(wq_s, bq_s, qT_s, 1, 512, 512),
                (2, 2, 7): lambda:
                    proj_qk_unit(wq_s, bq_s, qT_s, 1, 0, 512),
            }
            # output projections for chunk c right after head 3's chunk-c tail
            for c in range(NC):
                def op(c=c):
                    for tb in range(4 * c, 4 * c + 4):
                        for n in range(2):
                            oproj_unit(tb, n, last=(c == 0))
                fills[(3, c, 4 * c + 3)] = op

            proj_chunk(0)
            yts = {}
            for (h, c, j) in merged:
                if j == 0:
                    yts[h] = psY.tile([128, 512], F32, tag="yt", name=f"yt_{h}_{c}")
                qk_exp_av(h, c, j, yts[h])
                f = fills.pop((h, c, j), None)
                if f is not None:
                    f()
            assert not fills, f"unfired fill triggers: {list(fills)}"

    nc.compile()
    return nc


_NC = None


def _get_nc():
    global _NC
    if _NC is None:
        _NC = _build()
    return _NC


def kernel(x, Wq, bq, Wk, bk, Wv, bv, Wo, bo, _trace=False):
    x = np.asarray(x, dtype=np.float32)
    Wq = np.asarray(Wq, dtype=np.float32)
    Wk = np.asarray(Wk, dtype=np.float32)
    Wv = np.asarray(Wv, dtype=np.float32)
    Wo = np.asarray(Wo, dtype=np.float32)
    bq = np.asarray(bq, dtype=np.float32)
    bk = np.asarray(bk, dtype=np.float32)
    bv = np.asarray(bv, dtype=np.float32)
    bo = np.asarray(bo, dtype=np.float32)

    scale = np.float32(1.0 / np.sqrt(DH))
    bf = np.float16
    in_maps = []
    for c in range(8):
        b, roff = c // 4, (c % 4) * M
        in_maps.append({
            "xT": np.ascontiguousarray(x[b].T).astype(bf),
            "wqT": np.ascontiguousarray((Wq[roff:roff + M] * scale).T).astype(bf),
            "wkT": np.ascontiguousarray(Wk[roff:roff + M].T).astype(bf),
            "wvT": np.ascontiguousarray(Wv[roff:roff + M].T).astype(bf),
            "bq": np.ascontiguousarray(bq[roff:roff + M] * scale),
            "bk": np.ascontiguousarray(bk[roff:roff + M]),
            "bv": np.ascontiguousarray(bv[roff:roff + M]).astype(bf),
            "woT": np.ascontiguousarray(Wo[:, roff:roff + M].T).astype(bf),
        })

    nc = _get_nc()
    res = run_bass_kernel_spmd(nc, in_maps, list(range(8)), trace=_trace)

    out = np.empty((B, T, D), dtype=np.float32)
    for b in range(B):
        acc = np.zeros((T, D), dtype=np.float64)
        for c in range(4 * b, 4 * b + 4):
            acc += res.results[c]["outp"]
        out[b] = (acc + bo.astype(np.float64)).astype(np.float32)
    if _trace:
        kernel.last_results = res
    return out
